# revision 1
# baseline (speedup 1.0000x reference)
"""Trainium2 Bass kernel for nn_MatrixReasoner (segment_max over COO edges).

    contrib[k] = emb_vec[rows[k]] * vals[k]
    out[j]     = max(0, max_k { contrib[k] : cols[k] == j })

Strategy (8 NeuronCores, SPMD):
  - shard the 20M-edge COO list across the 8 cores (2.5M edges each),
    replicate the 1M-entry emb_vec;
  - per core: stream edge tiles [128, W]; gather emb via per-partition
    indirect DMA (one column of 128 edges per instruction); multiply by
    vals; scatter with DGE CCE *max* into a DRAM accumulator with 128
    partition-private interleaved copies (slot = col*128 + p).  Partition
    privacy kills cross-partition same-address RMW races; same-partition
    descriptors are generated and drained in order, so repeated (p, col)
    across instructions combine correctly via CCE max;
  - dense reduce of the 128 copies -> per-core partial [1M];
  - host max-reduces the 8 partials (the unshard step).

The DGE compute-op MAX is encoded by post-patching the NEFF: walrus only
maps bypass/add, but the cayman ISA + SWDGE ucode + SDMA CCE implement
MAX (0x03).  See cce_max_patch logic below.
"""

import io
import os
import sys
import tarfile
import tempfile

os.environ.setdefault("NEURON_SCRATCHPAD_PAGE_SIZE", "640")
sys.path.insert(0, "/opt/trn_rl_repo")

import numpy as np

from concourse import bass, bacc, mybir, tile
from concourse import bass_utils, bass2jax, neff as neff_mod

P = 128
N_ENT = 1_000_000
NNZ = 20_000_000
N_CORES = 8
COPIES = 128

N_PAD = 1 << 20            # table col space (pow2 >= N_ENT)
E_CORE = NNZ // N_CORES    # 2,500,000
NCOL_B = 2048              # columns per batch tile
NB = (E_CORE + P * NCOL_B - 1) // (P * NCOL_B)   # 10
NCOL_TOT = NB * NCOL_B     # 20480
E_PAD = P * NCOL_TOT       # 2,621,440

F32 = mybir.dt.float32
I32 = mybir.dt.int32

# ---------------------------------------------------------------------------
# NEFF patch: enable DGE compute_op=MAX (walrus only encodes bypass/add)
# ---------------------------------------------------------------------------

_orig_compile_bir_kernel = bass_utils.compile_bir_kernel
MAX_TOK = b'"cce_op":"max"'
ADD_TOK = b'"cce_op":"add"'
BYP_TOK = b'"cce_op":"bypass"'


def _untar_neff(neff_path, dst):
    with open(neff_path, "rb") as f:
        header = f.read(1024)
        with tarfile.open(fileobj=f, mode="r") as t:
            t.extractall(dst)
    return header


def _retar_neff(src_dir, old_header, out_path):
    buf = io.BytesIO()
    with tarfile.open(fileobj=buf, mode="w") as t:
        t.add(src_dir, arcname=".", filter=bass2jax._reset_tarinfo)
    data = buf.getvalue()
    new_header = neff_mod.make_deterministic_neff_header(
        old_neff_header=old_header, new_neff_data=data)
    with open(out_path, "wb") as f:
        f.write(new_header + data)


def _compile_bir_kernel_cce_max(bir_json, tmpdir, neff_name="file.neff"):
    n_max = bir_json.count(MAX_TOK)
    if n_max == 0:
        return _orig_compile_bir_kernel(bir_json, tmpdir, neff_name)
    j_add = bir_json.replace(MAX_TOK, ADD_TOK)
    j_byp = bir_json.replace(MAX_TOK, BYP_TOK)
    neff_add = _orig_compile_bir_kernel(j_add, tmpdir, neff_name)
    with tempfile.TemporaryDirectory() as td2:
        neff_byp = _orig_compile_bir_kernel(j_byp, td2, neff_name)
        da = tempfile.mkdtemp()
        db = tempfile.mkdtemp()
        header = _untar_neff(neff_add, da)
        _untar_neff(neff_byp, db)
    n_patched = 0
    for root, _dirs, files in os.walk(da):
        rel = os.path.relpath(root, da)
        for fn in files:
            fa = os.path.join(da, rel, fn)
            fb = os.path.join(db, rel, fn)
            if not fn.endswith(".bin") or not os.path.exists(fb):
                continue
            a = bytearray(open(fa, "rb").read())
            b = open(fb, "rb").read()
            if bytes(a) == b:
                continue
            assert len(a) == len(b), (fn, len(a), len(b))
            pos = [i for i in range(len(a)) if a[i] != b[i]]
            for i in pos:
                assert a[i] == 0x01 and b[i] == 0x00, (fn, i, a[i], b[i])
                a[i] = 0x03
            n_patched += len(pos)
            with open(fa, "wb") as f:
                f.write(bytes(a))
    assert n_patched == n_max, (n_patched, n_max)
    _retar_neff(da, header, neff_add)
    return neff_add


def _install_patch():
    bass_utils.compile_bir_kernel = _compile_bir_kernel_cce_max
    bass2jax.compile_bir_kernel = _compile_bir_kernel_cce_max


# ---------------------------------------------------------------------------
# Kernel builder
# ---------------------------------------------------------------------------

def build_nc():
    TBL = N_PAD * COPIES + P
    nc = bacc.Bacc("TRN2", target_bir_lowering=False, debug=False,
                   num_devices=N_CORES)
    emb_d = nc.dram_tensor("emb", (N_PAD,), F32, kind="ExternalInput").ap()
    rows_d = nc.dram_tensor("rows", (P, NCOL_TOT), I32, kind="ExternalInput").ap()
    cols_d = nc.dram_tensor("cols", (P, NCOL_TOT), I32, kind="ExternalInput").ap()
    vals_d = nc.dram_tensor("vals", (P, NCOL_TOT), F32, kind="ExternalInput").ap()
    out_d = nc.dram_tensor("out", (N_PAD,), F32, kind="ExternalOutput").ap()
    table = nc.dram_tensor("table", (TBL,), F32, kind="Internal").ap()
    emb2d = emb_d[:].rearrange("(n o) -> n o", o=1)
    tbl2d = table.rearrange("(n o) -> n o", o=1)

    with tile.TileContext(nc) as tc:
        with tc.tile_pool(name="z", bufs=1) as zp:
            zsb = zp.tile([P, 8192], F32)
            nc.vector.memset(zsb[:], 0.0)
            CH = P * 8192
            for i in range(TBL // CH):
                nc.sync.dma_start(
                    table[i * CH:(i + 1) * CH].rearrange("(p f) -> p f", p=P),
                    zsb[:])
            nc.sync.dma_start(
                table[(TBL // CH) * CH:].rearrange("(p f) -> p f", p=P),
                zsb[:, :1])

            with tc.tile_pool(name="m", bufs=2) as pool, \
                 tc.tile_pool(name="aux", bufs=1) as aux:
                iota_t = aux.tile([P, 1], I32)
                nc.gpsimd.iota(iota_t[:], pattern=[[0, 1]], base=0,
                               channel_multiplier=1)
                for b in range(NB):
                    cs = b * NCOL_B
                    ce = cs + NCOL_B
                    r_t = pool.tile([P, NCOL_B], I32, tag="r")
                    c_t = pool.tile([P, NCOL_B], I32, tag="c")
                    v_t = pool.tile([P, NCOL_B], F32, tag="v")
                    g_t = pool.tile([P, NCOL_B], F32, tag="g")
                    o_t = pool.tile([P, NCOL_B], I32, tag="o")
                    nc.sync.dma_start(r_t[:], rows_d[:, cs:ce])
                    nc.sync.dma_start(c_t[:], cols_d[:, cs:ce])
                    nc.sync.dma_start(v_t[:], vals_d[:, cs:ce])
                    for w in range(NCOL_B):
                        nc.gpsimd.indirect_dma_start(
                            out=g_t[:, w:w + 1], out_offset=None,
                            in_=emb2d,
                            in_offset=bass.IndirectOffsetOnAxis(
                                ap=r_t[:, w:w + 1], axis=0))
                    nc.vector.tensor_mul(out=g_t[:], in0=g_t[:], in1=v_t[:])
                    nc.vector.tensor_scalar(
                        out=o_t[:], in0=c_t[:], scalar1=7, scalar2=None,
                        op0=mybir.AluOpType.logical_shift_left)
                    nc.vector.tensor_tensor(
                        out=o_t[:], in0=o_t[:],
                        in1=iota_t[:, 0:1].to_broadcast([P, NCOL_B]),
                        op=mybir.AluOpType.bitwise_or)
                    for w in range(NCOL_B):
                        nc.gpsimd.indirect_dma_start(
                            out=tbl2d,
                            out_offset=bass.IndirectOffsetOnAxis(
                                ap=o_t[:, w:w + 1], axis=0),
                            in_=g_t[:, w:w + 1], in_offset=None,
                            compute_op=mybir.AluOpType.max)

            with tc.tile_pool(name="red", bufs=2) as rp:
                GT = 64
                TC_ = GT * P   # 8192 cols per reduce tile
                for t in range(N_PAD // TC_):
                    src = table[t * TC_ * COPIES:(t + 1) * TC_ * COPIES]
                    src = src.rearrange("(g p c) -> p g c", p=P, c=COPIES)
                    it = rp.tile([P, GT, COPIES], F32, tag="ri")
                    nc.sync.dma_start(it[:], src)
                    rt = rp.tile([P, GT], F32, tag="ro")
                    nc.vector.tensor_reduce(
                        out=rt[:], in_=it[:], axis=mybir.AxisListType.X,
                        op=mybir.AluOpType.max)
                    dst = out_d[t * TC_:(t + 1) * TC_].rearrange(
                        "(g p) -> p g", p=P)
                    nc.sync.dma_start(dst, rt[:])
    nc.compile()
    return nc


_nc_cache = None


def _get_nc():
    global _nc_cache
    if _nc_cache is None:
        _install_patch()
        _nc_cache = build_nc()
    return _nc_cache


def kernel(emb_vec, vals, rows, cols, rel_id=0):
    emb_vec = np.asarray(emb_vec, dtype=np.float32)
    vals = np.asarray(vals, dtype=np.float32)
    rows = np.asarray(rows, dtype=np.int32)
    cols = np.asarray(cols, dtype=np.int32)
    assert emb_vec.shape == (N_ENT,) and vals.shape == (NNZ,)

    emb_p = np.zeros(N_PAD, np.float32)
    emb_p[:N_ENT] = emb_vec

    nc = _get_nc()
    in_maps = []
    for c in range(N_CORES):
        sl = slice(c * E_CORE, (c + 1) * E_CORE)
        r = np.zeros(E_PAD, np.int32)
        cc = np.zeros(E_PAD, np.int32)
        v = np.zeros(E_PAD, np.float32)
        r[:E_CORE] = rows[sl]
        cc[:E_CORE] = cols[sl]
        v[:E_CORE] = vals[sl]   # pad edges: val 0 -> contrib 0, harmless
        in_maps.append({
            "emb": emb_p,
            "rows": r.reshape(P, NCOL_TOT),
            "cols": cc.reshape(P, NCOL_TOT),
            "vals": v.reshape(P, NCOL_TOT),
        })

    res = bass_utils.run_bass_kernel_spmd(
        nc, in_maps, core_ids=list(range(N_CORES)))
    partials = [np.asarray(res.results[c]["out"])[:N_ENT]
                for c in range(N_CORES)]
    out = np.maximum.reduce(partials)
    return np.maximum(out, np.float32(0.0))
